# revision 8
# baseline (speedup 1.0000x reference)
"""Trainium2 Bass kernel for nn_Attn_25417616458107 (sparse_attention).

Reference computation:
    energy[s,b,:] = enc[s,b,:] @ W^T + b_attn          # [S,B,H]
    score[b,s]    = hidden[0,b,:] . energy[s,b,:]       # [B,S]
    out           = softmax(score, axis=s)[:, None, :]  # [B,1,S]

Key algebraic reformulation: reassociating the two contractions,
    score[b,s] = (hidden[0,b,:] @ W) . enc[s,b,:] + hidden[0,b,:].b_attn
The bias term is constant per row b, so it cancels in the softmax.  With
q = hidden[0] @ W (a tiny [B,H]x[H,H] matmul done on the host), the device
kernel reduces to a batched dot-product stream over encoder_outputs plus a
row softmax -- memory-bound instead of the naive 275-GFLOP einsum.

Sharding: data-parallel over batch.  Each of the 8 cores gets 8 of the 64
batches: enc shard [S=2048, 8, H=1024] plus its q rows (pre-replicated to
the [128, 8, 1024] SBUF operand layout).  No cross-core communication.

On-chip layout per core:
    tile t (16 total) covers s in [128t, 128t+128), s = 128t + 8*sa + sb
    SBUF tile [partition=(b*16+sa), free=(sb,h)]  (4 MiB, 4KiB bursts)
    DVE: one tensor_tensor mult with q2 (in place)       16 x 8.7us
    ACT: per sb, Copy-activation with accum_out -> score  128 x 1.0us
    scores land in an SBUF [128, 128] tile; a DRAM bounce re-lays them as
    rows [b, s] for the softmax (max/exp+sum/scale) and the output DMA.
"""

import sys
import numpy as np

_S, _B, _H = 2048, 64, 1024
_NCORES = 8
_BLOC = _B // _NCORES  # 8 batches per core
_SA, _SB = 16, 8       # s = 128*t + 8*sa + sb; partition=(b,sa), free=(sb,h)
_NT = _S // (_SA * _SB)  # 16 tiles

_cache = {}


def _concourse():
    if "/opt/trn_rl_repo" not in sys.path:
        sys.path.insert(0, "/opt/trn_rl_repo")


def _build():
    _concourse()
    import concourse.bacc as bacc
    import concourse.mybir as mybir
    import concourse.tile as tile

    f32 = mybir.dt.float32
    nc = bacc.Bacc("TRN2", target_bir_lowering=False, debug=False)

    # enc is host-pre-transposed to [b, s, h] so each SBUF partition's
    # (sb, h) span is 32 KiB contiguous in DRAM -> full-size DMA packets.
    enc = nc.dram_tensor("enc", [_BLOC, _S, _H], f32, kind="ExternalInput")
    q2 = nc.dram_tensor("q2", [128, _SB, _H], f32, kind="ExternalInput")
    out = nc.dram_tensor("out", [_BLOC, _S], f32, kind="ExternalOutput")
    scratch = nc.dram_tensor("scratch", [128, _NT * _SB], f32)

    # tile t: partition p = b*16+sa, free f = sb*H+h  <->  enc[b, t*128 + sa*8 + sb, h]
    enc_r = enc.rearrange("b (t sa sb) h -> t b sa sb h", sa=_SA, sb=_SB)
    # scratch[b*16+sa, t*8+sb] -> rows[b, s] with s = t*128 + sa*8 + sb,
    # read back per-t (16 small DMAs) to stay within the 3-dim DMA AP limit.
    sc_rows = scratch.rearrange("(b sa) (t sb) -> t b sa sb", sa=_SA, t=_NT)

    with tile.TileContext(nc) as tc:
        with (
            tc.tile_pool(name="encp", bufs=3) as encp,
            tc.tile_pool(name="qp", bufs=1) as qp,
            tc.tile_pool(name="dumpp", bufs=2) as dumpp,
            tc.tile_pool(name="smallp", bufs=1) as smallp,
        ):
            q2t = qp.tile([128, _SB, _H], f32)
            nc.sync.dma_start(q2t[:], q2[:])

            scores = smallp.tile([128, _NT * _SB], f32)

            for t in range(_NT):
                et = encp.tile([128, _SB, _H], f32, tag="enc")
                nc.sync.dma_start(et[:], enc_r[t])
                nc.vector.tensor_mul(et[:], et[:], q2t[:])
                for sb in range(_SB):
                    dump = dumpp.tile([128, 1, _H], f32, tag="dump")
                    nc.scalar.activation(
                        dump[:],
                        et[:, sb : sb + 1, :],
                        mybir.ActivationFunctionType.Copy,
                        accum_out=scores[:, t * _SB + sb : t * _SB + sb + 1],
                    )

            nc.sync.dma_start(scratch[:], scores[:])
            rows = smallp.tile([_BLOC, _S], f32)
            rows_t = rows.rearrange("b (t sa sb) -> t b sa sb", t=_NT, sa=_SA)
            for t in range(_NT):
                nc.sync.dma_start(rows_t[t], sc_rows[t])

            negmx = smallp.tile([_BLOC, 1], f32)
            nc.vector.tensor_reduce(
                negmx[:],
                rows[:],
                axis=mybir.AxisListType.X,
                op=mybir.AluOpType.max,
                negate=True,
            )
            erows = smallp.tile([_BLOC, _S], f32)
            zsum = smallp.tile([_BLOC, 1], f32)
            nc.scalar.activation(
                erows[:],
                rows[:],
                mybir.ActivationFunctionType.Exp,
                bias=negmx[:],
                scale=1.0,
                accum_out=zsum[:],
            )
            rz = smallp.tile([_BLOC, 1], f32)
            nc.vector.reciprocal(rz[:], zsum[:])
            nc.vector.tensor_scalar_mul(erows[:], erows[:], rz[:])
            nc.sync.dma_start(out[:], erows[:])

    nc.compile()
    return nc


def _in_maps(hidden, encoder_outputs, W_attn):
    hidden = np.asarray(hidden, dtype=np.float32)
    enc = np.asarray(encoder_outputs, dtype=np.float32)
    W = np.asarray(W_attn, dtype=np.float32)
    q = hidden[0] @ W  # [B, H]; bias term is constant per row -> cancels in softmax
    maps = []
    for c in range(_NCORES):
        bsl = slice(c * _BLOC, (c + 1) * _BLOC)
        q2 = np.repeat(q[bsl], _SA, axis=0)  # [128, H], partition (b, sa)
        q2e = np.ascontiguousarray(
            np.broadcast_to(q2[:, None, :], (128, _SB, _H)), dtype=np.float32
        )
        maps.append(
            {
                "enc": np.ascontiguousarray(enc[:, bsl, :].transpose(1, 0, 2)),
                "q2": q2e,
            }
        )
    return maps


def kernel(hidden, encoder_outputs, W_attn, b_attn, **_unused):
    _concourse()
    from concourse.bass_utils import run_bass_kernel_spmd

    if "nc" not in _cache:
        _cache["nc"] = _build()
    nc = _cache["nc"]

    maps = _in_maps(hidden, encoder_outputs, W_attn)
    res = run_bass_kernel_spmd(nc, maps, core_ids=list(range(_NCORES)))
    outs = [np.asarray(res.results[c]["out"]) for c in range(_NCORES)]
    full = np.concatenate(outs, axis=0)  # [B, S]
    return full[:, None, :].astype(np.float32)


# revision 14
# speedup vs baseline: 3.1687x; 3.1687x over previous
"""Trainium2 Bass kernel for nn_Attn_25417616458107 (sparse_attention).

Reference computation:
    energy[s,b,:] = enc[s,b,:] @ W^T + b_attn          # [S,B,H]
    score[b,s]    = hidden[0,b,:] . energy[s,b,:]       # [B,S]
    out           = softmax(score, axis=s)[:, None, :]  # [B,1,S]

Key algebraic reformulation: reassociating the two contractions,
    score[b,s] = (hidden[0,b,:] @ W) . enc[s,b,:] + hidden[0,b,:].b_attn
The bias term is constant per row b, so it cancels in the softmax.  With
q = hidden[0] @ W (a tiny [B,H]x[H,H] matmul done on the host), the device
kernel reduces to a batched dot-product stream over encoder_outputs plus a
row softmax -- memory-bound instead of the naive 275-GFLOP einsum.

Sharding: data-parallel over batch.  Each of the 8 cores gets 8 of the 64
batches: enc shard [S=2048, 8, H=1024] plus its q rows (pre-replicated to
the [128, 8, 1024] SBUF operand layout).  No cross-core communication.

On-chip layout per core:
    tile t (16 total) covers s in [128t, 128t+128), s = 128t + 8*sa + sb
    SBUF tile [partition=(b*16+sa), free=(sb,h)]  (4 MiB, 4KiB bursts)
    DVE: one tensor_tensor mult with q2 (in place)       16 x 8.7us
    ACT: per sb, Copy-activation with accum_out -> score  128 x 1.0us
    scores land in an SBUF [128, 128] tile; a DRAM bounce re-lays them as
    rows [b, s] for the softmax (max/exp+sum/scale) and the output DMA.
"""

import sys
import numpy as np

_S, _B, _H = 2048, 64, 1024
_NCORES = 8
_BLOC = _B // _NCORES  # 8 batches per core
_SA, _SB = 16, 8       # s = 128*t + 8*sa + sb; partition=(b,sa), free=(sb,h)
_NT = _S // (_SA * _SB)  # 16 tiles

_cache = {}


def _concourse():
    if "/opt/trn_rl_repo" not in sys.path:
        sys.path.insert(0, "/opt/trn_rl_repo")


def _build():
    _concourse()
    import concourse.bacc as bacc
    import concourse.mybir as mybir
    import concourse.tile as tile

    f32 = mybir.dt.float32
    nc = bacc.Bacc("TRN2", target_bir_lowering=False, debug=False)

    # enc is host-pre-linearized to the exact tile layout [t, p, f]:
    # enc_lin[t, b*16+sa, sb*H+h] = enc[t*128+sa*8+sb, b, h].  Each tile is one
    # contiguous 4 MiB block, which is what lets the DMA engage all 16 SDMA
    # engines at line rate (~395 GB/s/core measured vs ~115 for strided APs).
    enc = nc.dram_tensor("enc", [_NT, 128, _SB * _H], f32, kind="ExternalInput")
    q2 = nc.dram_tensor("q2", [128, _SB * _H], f32, kind="ExternalInput")
    out = nc.dram_tensor("out", [_BLOC, _S], f32, kind="ExternalOutput")
    scratch = nc.dram_tensor("scratch", [128, _NT * _SB], f32)

    # scratch[b*16+sa, t*8+sb] -> rows[b, s] with s = t*128 + sa*8 + sb,
    # read back per-t (16 small DMAs) to stay within the 3-dim DMA AP limit.
    sc_rows = scratch.rearrange("(b sa) (t sb) -> t b sa sb", sa=_SA, t=_NT)

    with tile.TileContext(nc) as tc:
        with (
            tc.tile_pool(name="encp", bufs=3) as encp,
            tc.tile_pool(name="qp", bufs=1) as qp,
            tc.tile_pool(name="dumpp", bufs=2) as dumpp,
            tc.tile_pool(name="smallp", bufs=1) as smallp,
        ):
            q2t = qp.tile([128, _SB * _H], f32)
            nc.sync.dma_start(q2t[:], q2[:])

            scores = smallp.tile([128, _NT * _SB], f32)

            for t in range(_NT):
                et = encp.tile([128, _SB * _H], f32, tag="enc")
                nc.sync.dma_start(et[:], enc[t])
                nc.vector.tensor_mul(et[:], et[:], q2t[:])
                for sb in range(_SB):
                    dump = dumpp.tile([128, _H], f32, tag="dump")
                    nc.scalar.activation(
                        dump[:],
                        et[:, sb * _H : (sb + 1) * _H],
                        mybir.ActivationFunctionType.Copy,
                        accum_out=scores[:, t * _SB + sb : t * _SB + sb + 1],
                    )

            nc.sync.dma_start(scratch[:], scores[:])
            rows = smallp.tile([_BLOC, _S], f32)
            rows_t = rows.rearrange("b (t sa sb) -> t b sa sb", t=_NT, sa=_SA)
            for t in range(_NT):
                nc.sync.dma_start(rows_t[t], sc_rows[t])

            negmx = smallp.tile([_BLOC, 1], f32)
            nc.vector.tensor_reduce(
                negmx[:],
                rows[:],
                axis=mybir.AxisListType.X,
                op=mybir.AluOpType.max,
                negate=True,
            )
            erows = smallp.tile([_BLOC, _S], f32)
            zsum = smallp.tile([_BLOC, 1], f32)
            nc.scalar.activation(
                erows[:],
                rows[:],
                mybir.ActivationFunctionType.Exp,
                bias=negmx[:],
                scale=1.0,
                accum_out=zsum[:],
            )
            rz = smallp.tile([_BLOC, 1], f32)
            nc.vector.reciprocal(rz[:], zsum[:])
            nc.vector.tensor_scalar_mul(erows[:], erows[:], rz[:])
            nc.sync.dma_start(out[:], erows[:])

    nc.compile()
    return nc


def _in_maps(hidden, encoder_outputs, W_attn):
    hidden = np.asarray(hidden, dtype=np.float32)
    enc = np.asarray(encoder_outputs, dtype=np.float32)
    W = np.asarray(W_attn, dtype=np.float32)
    q = hidden[0] @ W  # [B, H]; bias term is constant per row -> cancels in softmax
    maps = []
    for c in range(_NCORES):
        bsl = slice(c * _BLOC, (c + 1) * _BLOC)
        q2 = np.repeat(q[bsl], _SA, axis=0)  # [128, H], partition p = b*16+sa
        q2e = np.ascontiguousarray(
            np.broadcast_to(q2[:, None, :], (128, _SB, _H)), dtype=np.float32
        ).reshape(128, _SB * _H)
        # linearize the shard into the exact on-chip tile layout [t, p, f]
        enc_lin = np.ascontiguousarray(
            enc[:, bsl, :]
            .reshape(_NT, _SA, _SB, _BLOC, _H)  # t, sa, sb, b, h
            .transpose(0, 3, 1, 2, 4)           # t, b, sa, sb, h
            .reshape(_NT, 128, _SB * _H)
        )
        maps.append({"enc": enc_lin, "q2": q2e})
    return maps


def kernel(hidden, encoder_outputs, W_attn, b_attn, **_unused):
    _concourse()
    from concourse.bass_utils import run_bass_kernel_spmd

    if "nc" not in _cache:
        _cache["nc"] = _build()
    nc = _cache["nc"]

    maps = _in_maps(hidden, encoder_outputs, W_attn)
    res = run_bass_kernel_spmd(nc, maps, core_ids=list(range(_NCORES)))
    outs = [np.asarray(res.results[c]["out"]) for c in range(_NCORES)]
    full = np.concatenate(outs, axis=0)  # [B, S]
    return full[:, None, :].astype(np.float32)


# revision 16
# speedup vs baseline: 3.2527x; 1.0265x over previous
"""Trainium2 Bass kernel for nn_Attn_25417616458107 (sparse_attention).

Reference computation:
    energy[s,b,:] = enc[s,b,:] @ W^T + b_attn          # [S,B,H]
    score[b,s]    = hidden[0,b,:] . energy[s,b,:]       # [B,S]
    out           = softmax(score, axis=s)[:, None, :]  # [B,1,S]

Key algebraic reformulation: reassociating the two contractions,
    score[b,s] = (hidden[0,b,:] @ W) . enc[s,b,:] + hidden[0,b,:].b_attn
The bias term is constant per row b, so it cancels in the softmax.  With
q = hidden[0] @ W (a tiny [B,H]x[H,H] matmul done on the host), the device
kernel reduces to a batched dot-product stream over encoder_outputs plus a
row softmax -- memory-bound instead of the naive 275-GFLOP einsum.

Sharding: data-parallel over batch.  Each of the 8 cores gets 8 of the 64
batches.  No cross-core communication.

Per core: 16 tiles, tile t covers s in [128t, 128t+128), s = 128t + 8*sa + sb.
SBUF tile [partition p=(b*16+sa), free f=(sb,h)].  The host pre-linearizes the
enc shard into exactly this [t, p, f] layout so every tile is one contiguous
4 MiB DMA (engages all 16 SDMA engines at ~395 GB/s/core; strided APs only
reached ~115).  Compute is a single fused DVE op per (tile, sb):
tensor_tensor_reduce does et*q2 (in place) + free-dim sum -> score column.
Scores bounce through DRAM (per tile, pipelined) to re-lay them as rows
[b, s] for the softmax (max / exp+sum via ACT accum / scale) and output DMA.
"""

import sys
import numpy as np

_S, _B, _H = 2048, 64, 1024
_NCORES = 8
_BLOC = _B // _NCORES  # 8 batches per core
_SA, _SB = 16, 8       # s = 128*t + 8*sa + sb; partition p = b*16+sa
_NT = _S // (_SA * _SB)  # 16 tiles

_cache = {}


def _concourse():
    if "/opt/trn_rl_repo" not in sys.path:
        sys.path.insert(0, "/opt/trn_rl_repo")


def _build():
    _concourse()
    import concourse.bacc as bacc
    import concourse.mybir as mybir
    import concourse.tile as tile

    f32 = mybir.dt.float32
    nc = bacc.Bacc("TRN2", target_bir_lowering=False, debug=False)

    enc = nc.dram_tensor("enc", [_NT, 128, _SB * _H], f32, kind="ExternalInput")
    q2 = nc.dram_tensor("q2", [128, _H], f32, kind="ExternalInput")
    out = nc.dram_tensor("out", [_BLOC, _S], f32, kind="ExternalOutput")
    scratch = nc.dram_tensor("scratch", [128, _NT * _SB], f32)

    # scratch[b*16+sa, t*8+sb] -> rows[b, s] with s = t*128 + sa*8 + sb,
    # bounced per-t (small DMAs) to stay within the 3-dim DMA AP limit.
    sc_cols = scratch.rearrange("p (t sb) -> t p sb", t=_NT)
    sc_rows = scratch.rearrange("(b sa) (t sb) -> t b sa sb", sa=_SA, t=_NT)

    with tile.TileContext(nc) as tc:
        with (
            tc.tile_pool(name="encp", bufs=4) as encp,
            tc.tile_pool(name="qp", bufs=1) as qp,
            tc.tile_pool(name="dumpp", bufs=2) as dumpp,
            tc.tile_pool(name="smallp", bufs=1) as smallp,
        ):
            q2t = qp.tile([128, _H], f32)
            nc.sync.dma_start(q2t[:], q2[:])

            scores = smallp.tile([128, _NT * _SB], f32)
            rows = smallp.tile([_BLOC, _S], f32)
            rows_t = rows.rearrange("b (t sa sb) -> t b sa sb", t=_NT, sa=_SA)

            for t in range(_NT):
                et = encp.tile([128, _SB * _H], f32, tag="enc")
                nc.sync.dma_start(et[:], enc[t])
                for sb in range(_SB):
                    sl = slice(sb * _H, (sb + 1) * _H)
                    # in-place product on DVE, free-dim sum on ACT's accum port
                    nc.vector.tensor_mul(et[:, sl], et[:, sl], q2t[:])
                    dump = dumpp.tile([128, _H], f32, tag="dump")
                    nc.scalar.activation(
                        dump[:],
                        et[:, sl],
                        mybir.ActivationFunctionType.Copy,
                        accum_out=scores[:, t * _SB + sb : t * _SB + sb + 1],
                    )
                # pipelined bounce: tile t's score columns -> DRAM -> row layout
                nc.sync.dma_start(sc_cols[t], scores[:, t * _SB : (t + 1) * _SB])
                nc.sync.dma_start(rows_t[t], sc_rows[t])

            negmx = smallp.tile([_BLOC, 1], f32)
            nc.vector.tensor_reduce(
                negmx[:],
                rows[:],
                axis=mybir.AxisListType.X,
                op=mybir.AluOpType.max,
                negate=True,
            )
            erows = smallp.tile([_BLOC, _S], f32)
            zsum = smallp.tile([_BLOC, 1], f32)
            nc.scalar.activation(
                erows[:],
                rows[:],
                mybir.ActivationFunctionType.Exp,
                bias=negmx[:],
                scale=1.0,
                accum_out=zsum[:],
            )
            rz = smallp.tile([_BLOC, 1], f32)
            nc.vector.reciprocal(rz[:], zsum[:])
            nc.vector.tensor_scalar_mul(erows[:], erows[:], rz[:])
            nc.sync.dma_start(out[:], erows[:])

    nc.compile()
    return nc


def _in_maps(hidden, encoder_outputs, W_attn):
    hidden = np.asarray(hidden, dtype=np.float32)
    enc = np.asarray(encoder_outputs, dtype=np.float32)
    W = np.asarray(W_attn, dtype=np.float32)
    q = hidden[0] @ W  # [B, H]; bias term is constant per row -> cancels in softmax
    maps = []
    for c in range(_NCORES):
        bsl = slice(c * _BLOC, (c + 1) * _BLOC)
        q2 = np.ascontiguousarray(np.repeat(q[bsl], _SA, axis=0))  # [128, H]
        # linearize the shard into the exact on-chip tile layout [t, p, f]
        enc_lin = np.ascontiguousarray(
            enc[:, bsl, :]
            .reshape(_NT, _SA, _SB, _BLOC, _H)  # t, sa, sb, b, h
            .transpose(0, 3, 1, 2, 4)           # t, b, sa, sb, h
            .reshape(_NT, 128, _SB * _H)
        )
        maps.append({"enc": enc_lin, "q2": q2})
    return maps


def kernel(hidden, encoder_outputs, W_attn, b_attn, **_unused):
    _concourse()
    from concourse.bass_utils import run_bass_kernel_spmd

    if "nc" not in _cache:
        _cache["nc"] = _build()
    nc = _cache["nc"]

    maps = _in_maps(hidden, encoder_outputs, W_attn)
    res = run_bass_kernel_spmd(nc, maps, core_ids=list(range(_NCORES)))
    outs = [np.asarray(res.results[c]["out"]) for c in range(_NCORES)]
    full = np.concatenate(outs, axis=0)  # [B, S]
    return full[:, None, :].astype(np.float32)


# revision 18
# speedup vs baseline: 3.4632x; 1.0647x over previous
"""Trainium2 Bass kernel for nn_Attn_25417616458107 (sparse_attention).

Reference computation:
    energy[s,b,:] = enc[s,b,:] @ W^T + b_attn          # [S,B,H]
    score[b,s]    = hidden[0,b,:] . energy[s,b,:]       # [B,S]
    out           = softmax(score, axis=s)[:, None, :]  # [B,1,S]

Key algebraic reformulation: reassociating the two contractions,
    score[b,s] = (hidden[0,b,:] @ W) . enc[s,b,:] + hidden[0,b,:].b_attn
The bias term is constant per row b, so it cancels in the softmax.  With
q = hidden[0] @ W (a tiny [B,H]x[H,H] matmul done on the host), the device
kernel reduces to a batched dot-product stream over encoder_outputs plus a
row softmax -- memory-bound instead of the naive 275-GFLOP einsum.

Sharding: data-parallel over batch.  Each of the 8 cores gets 8 of the 64
batches.  No cross-core communication.

Per core: 16 tiles, tile t covers s in [128t, 128t+128), s = 128t + 8*sa + sb.
SBUF tile [partition p=(b*16+sa), free f=(sb,h)].  The host pre-linearizes the
enc shard into exactly this [t, p, f] layout so every tile is one contiguous
4 MiB DMA (engages all 16 SDMA engines at ~395 GB/s/core; strided APs only
reached ~115).  Compute is a single fused DVE op per (tile, sb):
tensor_tensor_reduce does et*q2 (in place) + free-dim sum -> score column.
Scores bounce through DRAM (per tile, pipelined) to re-lay them as rows
[b, s] for the softmax (max / exp+sum via ACT accum / scale) and output DMA.
"""

import sys
import numpy as np

_S, _B, _H = 2048, 64, 1024
_NCORES = 8
_BLOC = _B // _NCORES  # 8 batches per core
_SA, _SB = 16, 8       # s = 128*t + 8*sa + sb; partition p = b*16+sa
_NT = _S // (_SA * _SB)  # 16 tiles

_cache = {}


def _concourse():
    if "/opt/trn_rl_repo" not in sys.path:
        sys.path.insert(0, "/opt/trn_rl_repo")


def _build():
    _concourse()
    import concourse.bacc as bacc
    import concourse.mybir as mybir
    import concourse.tile as tile

    f32 = mybir.dt.float32
    nc = bacc.Bacc("TRN2", target_bir_lowering=False, debug=False)

    enc = nc.dram_tensor("enc", [_NT, 128, _SB * _H], f32, kind="ExternalInput")
    q2 = nc.dram_tensor("q2", [128, _H], f32, kind="ExternalInput")
    out = nc.dram_tensor("out", [_BLOC, _S], f32, kind="ExternalOutput")
    scratch = nc.dram_tensor("scratch", [128, _NT * _SB], f32)

    # scratch[b*16+sa, t*8+sb] -> rows[b, s] with s = t*128 + sa*8 + sb,
    # bounced per-t (small DMAs) to stay within the 3-dim DMA AP limit.
    sc_cols = scratch.rearrange("p (t sb) -> t p sb", t=_NT)
    sc_rows = scratch.rearrange("(b sa) (t sb) -> t b sa sb", sa=_SA, t=_NT)

    with tile.TileContext(nc) as tc:
        with (
            tc.tile_pool(name="encp", bufs=5) as encp,
            tc.tile_pool(name="qp", bufs=1) as qp,
            tc.tile_pool(name="dumpp", bufs=2) as dumpp,
            tc.tile_pool(name="smallp", bufs=1) as smallp,
        ):
            q2t = qp.tile([128, _H], f32)
            nc.sync.dma_start(q2t[:], q2[:])

            scores = smallp.tile([128, _NT * _SB], f32)
            rows = smallp.tile([_BLOC, _S], f32)
            rows_t = rows.rearrange("b (t sa sb) -> t b sa sb", t=_NT, sa=_SA)

            for t in range(_NT):
                et = encp.tile([128, _SB * _H], f32, tag="enc")
                nc.sync.dma_start(et[:], enc[t])
                for sb in range(_SB):
                    sl = slice(sb * _H, (sb + 1) * _H)
                    # in-place product on DVE, free-dim sum on ACT's accum port
                    nc.vector.tensor_mul(et[:, sl], et[:, sl], q2t[:])
                    dump = dumpp.tile([128, _H], f32, tag="dump")
                    nc.scalar.activation(
                        dump[:],
                        et[:, sl],
                        mybir.ActivationFunctionType.Copy,
                        accum_out=scores[:, t * _SB + sb : t * _SB + sb + 1],
                    )
                # pipelined bounce: tile t's score columns -> DRAM -> row layout.
                # On GPSIMD's SWDGE ring so its semaphore waits never block the
                # SP ring that streams enc tiles (head-of-line blocking).
                nc.gpsimd.dma_start(sc_cols[t], scores[:, t * _SB : (t + 1) * _SB])
                nc.gpsimd.dma_start(rows_t[t], sc_rows[t])

            negmx = smallp.tile([_BLOC, 1], f32)
            nc.vector.tensor_reduce(
                negmx[:],
                rows[:],
                axis=mybir.AxisListType.X,
                op=mybir.AluOpType.max,
                negate=True,
            )
            erows = smallp.tile([_BLOC, _S], f32)
            zsum = smallp.tile([_BLOC, 1], f32)
            nc.scalar.activation(
                erows[:],
                rows[:],
                mybir.ActivationFunctionType.Exp,
                bias=negmx[:],
                scale=1.0,
                accum_out=zsum[:],
            )
            rz = smallp.tile([_BLOC, 1], f32)
            nc.vector.reciprocal(rz[:], zsum[:])
            nc.vector.tensor_scalar_mul(erows[:], erows[:], rz[:])
            nc.sync.dma_start(out[:], erows[:])

    nc.compile()
    return nc


def _in_maps(hidden, encoder_outputs, W_attn):
    hidden = np.asarray(hidden, dtype=np.float32)
    enc = np.asarray(encoder_outputs, dtype=np.float32)
    W = np.asarray(W_attn, dtype=np.float32)
    q = hidden[0] @ W  # [B, H]; bias term is constant per row -> cancels in softmax
    maps = []
    for c in range(_NCORES):
        bsl = slice(c * _BLOC, (c + 1) * _BLOC)
        q2 = np.ascontiguousarray(np.repeat(q[bsl], _SA, axis=0))  # [128, H]
        # linearize the shard into the exact on-chip tile layout [t, p, f]
        enc_lin = np.ascontiguousarray(
            enc[:, bsl, :]
            .reshape(_NT, _SA, _SB, _BLOC, _H)  # t, sa, sb, b, h
            .transpose(0, 3, 1, 2, 4)           # t, b, sa, sb, h
            .reshape(_NT, 128, _SB * _H)
        )
        maps.append({"enc": enc_lin, "q2": q2})
    return maps


def kernel(hidden, encoder_outputs, W_attn, b_attn, **_unused):
    _concourse()
    from concourse.bass_utils import run_bass_kernel_spmd

    if "nc" not in _cache:
        _cache["nc"] = _build()
    nc = _cache["nc"]

    maps = _in_maps(hidden, encoder_outputs, W_attn)
    res = run_bass_kernel_spmd(nc, maps, core_ids=list(range(_NCORES)))
    outs = [np.asarray(res.results[c]["out"]) for c in range(_NCORES)]
    full = np.concatenate(outs, axis=0)  # [B, S]
    return full[:, None, :].astype(np.float32)


# revision 19
# speedup vs baseline: 3.5020x; 1.0112x over previous
"""Trainium2 Bass kernel for nn_Attn_25417616458107 (sparse_attention).

Reference computation:
    energy[s,b,:] = enc[s,b,:] @ W^T + b_attn          # [S,B,H]
    score[b,s]    = hidden[0,b,:] . energy[s,b,:]       # [B,S]
    out           = softmax(score, axis=s)[:, None, :]  # [B,1,S]

Key algebraic reformulation: reassociating the two contractions,
    score[b,s] = (hidden[0,b,:] @ W) . enc[s,b,:] + hidden[0,b,:].b_attn
The bias term is constant per row b, so it cancels in the softmax.  With
q = hidden[0] @ W (a tiny [B,H]x[H,H] matmul done on the host), the device
kernel reduces to a batched dot-product stream over encoder_outputs plus a
row softmax -- memory-bound instead of the naive 275-GFLOP einsum.

Sharding: data-parallel over batch.  Each of the 8 cores gets 8 of the 64
batches.  No cross-core communication.

Per core: 16 tiles, tile t covers s in [128t, 128t+128), s = 128t + 8*sa + sb.
SBUF tile [partition p=(b*16+sa), free f=(sb,h)].  The host pre-linearizes the
enc shard into exactly this [t, p, f] layout so every tile is one contiguous
4 MiB DMA (engages all 16 SDMA engines at ~395 GB/s/core; strided APs only
reached ~115).  Compute is a single fused DVE op per (tile, sb):
tensor_tensor_reduce does et*q2 (in place) + free-dim sum -> score column.
Scores bounce through DRAM (per tile, pipelined) to re-lay them as rows
[b, s] for the softmax (max / exp+sum via ACT accum / scale) and output DMA.
"""

import sys
import numpy as np

_S, _B, _H = 2048, 64, 1024
_NCORES = 8
_BLOC = _B // _NCORES  # 8 batches per core
_SA, _SB = 16, 8       # s = 128*t + 8*sa + sb; partition p = b*16+sa
_NT = _S // (_SA * _SB)  # 16 tiles

_cache = {}


def _concourse():
    if "/opt/trn_rl_repo" not in sys.path:
        sys.path.insert(0, "/opt/trn_rl_repo")


def _build():
    _concourse()
    import concourse.bacc as bacc
    import concourse.mybir as mybir
    import concourse.tile as tile

    f32 = mybir.dt.float32
    nc = bacc.Bacc("TRN2", target_bir_lowering=False, debug=False)

    enc = nc.dram_tensor("enc", [_NT, 128, _SB * _H], f32, kind="ExternalInput")
    q2 = nc.dram_tensor("q2", [128, _H], f32, kind="ExternalInput")
    out = nc.dram_tensor("out", [_BLOC, _S], f32, kind="ExternalOutput")
    scratch = nc.dram_tensor("scratch", [128, _NT * _SB], f32)

    # scratch[b*16+sa, t*8+sb] -> rows[b, s] with s = t*128 + sa*8 + sb,
    # bounced per-t (small DMAs) to stay within the 3-dim DMA AP limit.
    sc_cols = scratch.rearrange("p (t sb) -> t p sb", t=_NT)
    sc_rows = scratch.rearrange("(b sa) (t sb) -> t b sa sb", sa=_SA, t=_NT)

    with tile.TileContext(nc) as tc:
        with (
            tc.tile_pool(name="encp", bufs=5) as encp,
            tc.tile_pool(name="qp", bufs=1) as qp,
            tc.tile_pool(name="dumpp", bufs=2) as dumpp,
            tc.tile_pool(name="smallp", bufs=1) as smallp,
        ):
            q2t = qp.tile([128, _H], f32)
            nc.sync.dma_start(q2t[:], q2[:])

            scores = smallp.tile([128, _NT * _SB], f32)
            rows = smallp.tile([_BLOC, _S], f32)
            rows_t = rows.rearrange("b (t sa sb) -> t b sa sb", t=_NT, sa=_SA)

            for t in range(_NT):
                et = encp.tile([128, _SB * _H], f32, tag="enc")
                nc.sync.dma_start(et[:], enc[t])
                for sb in range(_SB):
                    sl = slice(sb * _H, (sb + 1) * _H)
                    # in-place product on DVE, free-dim sum on ACT's accum port
                    nc.vector.tensor_mul(et[:, sl], et[:, sl], q2t[:])
                    dump = dumpp.tile([128, _H], f32, tag="dump")
                    nc.scalar.activation(
                        dump[:],
                        et[:, sl],
                        mybir.ActivationFunctionType.Copy,
                        accum_out=scores[:, t * _SB + sb : t * _SB + sb + 1],
                    )
                # pipelined bounce out: tile t's score columns -> DRAM, issued on
                # the ACT HWDGE ring (its wait is satisfied by ACT program order,
                # so it never stalls; SP ring stays a pure enc stream; SWDGE
                # would drag SDMA engines 7/15 via descriptor-ring port traffic).
                nc.scalar.dma_start(sc_cols[t], scores[:, t * _SB : (t + 1) * _SB])

            # bounce back in: re-lay scores as rows[b, s]; all cols landed long ago
            for t in range(_NT):
                nc.sync.dma_start(rows_t[t], sc_rows[t])

            negmx = smallp.tile([_BLOC, 1], f32)
            nc.vector.tensor_reduce(
                negmx[:],
                rows[:],
                axis=mybir.AxisListType.X,
                op=mybir.AluOpType.max,
                negate=True,
            )
            erows = smallp.tile([_BLOC, _S], f32)
            zsum = smallp.tile([_BLOC, 1], f32)
            nc.scalar.activation(
                erows[:],
                rows[:],
                mybir.ActivationFunctionType.Exp,
                bias=negmx[:],
                scale=1.0,
                accum_out=zsum[:],
            )
            rz = smallp.tile([_BLOC, 1], f32)
            nc.vector.reciprocal(rz[:], zsum[:])
            nc.vector.tensor_scalar_mul(erows[:], erows[:], rz[:])
            nc.sync.dma_start(out[:], erows[:])

    nc.compile()
    return nc


def _in_maps(hidden, encoder_outputs, W_attn):
    hidden = np.asarray(hidden, dtype=np.float32)
    enc = np.asarray(encoder_outputs, dtype=np.float32)
    W = np.asarray(W_attn, dtype=np.float32)
    q = hidden[0] @ W  # [B, H]; bias term is constant per row -> cancels in softmax
    maps = []
    for c in range(_NCORES):
        bsl = slice(c * _BLOC, (c + 1) * _BLOC)
        q2 = np.ascontiguousarray(np.repeat(q[bsl], _SA, axis=0))  # [128, H]
        # linearize the shard into the exact on-chip tile layout [t, p, f]
        enc_lin = np.ascontiguousarray(
            enc[:, bsl, :]
            .reshape(_NT, _SA, _SB, _BLOC, _H)  # t, sa, sb, b, h
            .transpose(0, 3, 1, 2, 4)           # t, b, sa, sb, h
            .reshape(_NT, 128, _SB * _H)
        )
        maps.append({"enc": enc_lin, "q2": q2})
    return maps


def kernel(hidden, encoder_outputs, W_attn, b_attn, **_unused):
    _concourse()
    from concourse.bass_utils import run_bass_kernel_spmd

    if "nc" not in _cache:
        _cache["nc"] = _build()
    nc = _cache["nc"]

    maps = _in_maps(hidden, encoder_outputs, W_attn)
    res = run_bass_kernel_spmd(nc, maps, core_ids=list(range(_NCORES)))
    outs = [np.asarray(res.results[c]["out"]) for c in range(_NCORES)]
    full = np.concatenate(outs, axis=0)  # [B, S]
    return full[:, None, :].astype(np.float32)


# revision 20
# speedup vs baseline: 4.3818x; 1.2512x over previous
"""Trainium2 Bass kernel for nn_Attn_25417616458107 (sparse_attention).

Reference computation:
    energy[s,b,:] = enc[s,b,:] @ W^T + b_attn          # [S,B,H]
    score[b,s]    = hidden[0,b,:] . energy[s,b,:]       # [B,S]
    out           = softmax(score, axis=s)[:, None, :]  # [B,1,S]

Key algebraic reformulation: reassociating the two contractions,
    score[b,s] = (hidden[0,b,:] @ W) . enc[s,b,:] + hidden[0,b,:].b_attn
The bias term is constant per row b, so it cancels in the softmax.  With
q = hidden[0] @ W (a tiny [B,H]x[H,H] matmul done on the host), the device
kernel reduces to a batched dot-product stream over encoder_outputs plus a
row softmax -- memory-bound instead of the naive 275-GFLOP einsum.

Sharding: data-parallel over batch.  Each of the 8 cores gets 8 of the 64
batches.  No cross-core communication.

Per core: 16 tiles, tile t covers s in [128t, 128t+128), s = 128t + 8*sa + sb.
SBUF tile [partition p=(b*16+sa), free f=(sb,h)].  The host pre-linearizes the
enc shard into exactly this [t, p, f] layout so every tile is one contiguous
4 MiB DMA (engages all 16 SDMA engines at ~395 GB/s/core; strided APs only
reached ~115).  Compute is a single fused DVE op per (tile, sb):
tensor_tensor_reduce does et*q2 (in place) + free-dim sum -> score column.
Scores bounce through DRAM (per tile, pipelined) to re-lay them as rows
[b, s] for the softmax (max / exp+sum via ACT accum / scale) and output DMA.
"""

import sys
import numpy as np

_S, _B, _H = 2048, 64, 1024
_NCORES = 8
_BLOC = _B // _NCORES  # 8 batches per core
_SA, _SB = 16, 8       # s = 128*t + 8*sa + sb; partition p = b*16+sa
_NT = _S // (_SA * _SB)  # 16 tiles

_cache = {}


def _concourse():
    if "/opt/trn_rl_repo" not in sys.path:
        sys.path.insert(0, "/opt/trn_rl_repo")


def _build():
    _concourse()
    import concourse.bacc as bacc
    import concourse.mybir as mybir
    import concourse.tile as tile

    f32 = mybir.dt.float32
    f16 = mybir.dt.float16
    nc = bacc.Bacc("TRN2", target_bir_lowering=False, debug=False)

    # enc/q2 staged in fp16: halves the HBM stream (the kernel's binding
    # resource) and enables the DVE 2x_1P perf mode for the multiply.
    # Scores accumulate in fp32; measured end-to-end rel err ~1.8e-3.
    enc = nc.dram_tensor("enc", [_NT, 128, _SB * _H], f16, kind="ExternalInput")
    q2 = nc.dram_tensor("q2", [128, _H], f16, kind="ExternalInput")
    out = nc.dram_tensor("out", [_BLOC, _S], f32, kind="ExternalOutput")
    scratch = nc.dram_tensor("scratch", [128, _NT * _SB], f32)

    # scratch[b*16+sa, t*8+sb] -> rows[b, s] with s = t*128 + sa*8 + sb,
    # bounced per-t (small DMAs) to stay within the 3-dim DMA AP limit.
    sc_cols = scratch.rearrange("p (t sb) -> t p sb", t=_NT)
    sc_rows = scratch.rearrange("(b sa) (t sb) -> t b sa sb", sa=_SA, t=_NT)

    with tile.TileContext(nc) as tc:
        with (
            tc.tile_pool(name="encp", bufs=5) as encp,
            tc.tile_pool(name="qp", bufs=1) as qp,
            tc.tile_pool(name="dumpp", bufs=2) as dumpp,
            tc.tile_pool(name="smallp", bufs=1) as smallp,
        ):
            q2t = qp.tile([128, _H], f16)
            nc.sync.dma_start(q2t[:], q2[:])

            scores = smallp.tile([128, _NT * _SB], f32)
            rows = smallp.tile([_BLOC, _S], f32)
            rows_t = rows.rearrange("b (t sa sb) -> t b sa sb", t=_NT, sa=_SA)

            for t in range(_NT):
                et = encp.tile([128, _SB * _H], f16, tag="enc")
                nc.sync.dma_start(et[:], enc[t])
                for sb in range(_SB):
                    sl = slice(sb * _H, (sb + 1) * _H)
                    # in-place product on DVE, free-dim sum on ACT's accum port
                    nc.vector.tensor_mul(et[:, sl], et[:, sl], q2t[:])
                    dump = dumpp.tile([128, _H], f16, tag="dump")
                    nc.scalar.activation(
                        dump[:],
                        et[:, sl],
                        mybir.ActivationFunctionType.Copy,
                        accum_out=scores[:, t * _SB + sb : t * _SB + sb + 1],
                    )
                # pipelined bounce out: tile t's score columns -> DRAM, issued on
                # the ACT HWDGE ring (its wait is satisfied by ACT program order,
                # so it never stalls; SP ring stays a pure enc stream; SWDGE
                # would drag SDMA engines 7/15 via descriptor-ring port traffic).
                nc.scalar.dma_start(sc_cols[t], scores[:, t * _SB : (t + 1) * _SB])

            # bounce back in: re-lay scores as rows[b, s]; all cols landed long ago
            for t in range(_NT):
                nc.sync.dma_start(rows_t[t], sc_rows[t])

            negmx = smallp.tile([_BLOC, 1], f32)
            nc.vector.tensor_reduce(
                negmx[:],
                rows[:],
                axis=mybir.AxisListType.X,
                op=mybir.AluOpType.max,
                negate=True,
            )
            erows = smallp.tile([_BLOC, _S], f32)
            zsum = smallp.tile([_BLOC, 1], f32)
            nc.scalar.activation(
                erows[:],
                rows[:],
                mybir.ActivationFunctionType.Exp,
                bias=negmx[:],
                scale=1.0,
                accum_out=zsum[:],
            )
            rz = smallp.tile([_BLOC, 1], f32)
            nc.vector.reciprocal(rz[:], zsum[:])
            nc.vector.tensor_scalar_mul(erows[:], erows[:], rz[:])
            nc.sync.dma_start(out[:], erows[:])

    nc.compile()
    return nc


def _in_maps(hidden, encoder_outputs, W_attn):
    hidden = np.asarray(hidden, dtype=np.float32)
    enc = np.asarray(encoder_outputs, dtype=np.float32)
    W = np.asarray(W_attn, dtype=np.float32)
    q = hidden[0] @ W  # [B, H]; bias term is constant per row -> cancels in softmax
    maps = []
    for c in range(_NCORES):
        bsl = slice(c * _BLOC, (c + 1) * _BLOC)
        q2 = np.ascontiguousarray(np.repeat(q[bsl], _SA, axis=0), dtype=np.float16)
        # linearize the shard into the exact on-chip tile layout [t, p, f]
        enc_lin = np.ascontiguousarray(
            enc[:, bsl, :]
            .reshape(_NT, _SA, _SB, _BLOC, _H)  # t, sa, sb, b, h
            .transpose(0, 3, 1, 2, 4)           # t, b, sa, sb, h
            .reshape(_NT, 128, _SB * _H)
            .astype(np.float16)
        )
        maps.append({"enc": enc_lin, "q2": q2})
    return maps


def kernel(hidden, encoder_outputs, W_attn, b_attn, **_unused):
    _concourse()
    from concourse.bass_utils import run_bass_kernel_spmd

    if "nc" not in _cache:
        _cache["nc"] = _build()
    nc = _cache["nc"]

    maps = _in_maps(hidden, encoder_outputs, W_attn)
    res = run_bass_kernel_spmd(nc, maps, core_ids=list(range(_NCORES)))
    outs = [np.asarray(res.results[c]["out"]) for c in range(_NCORES)]
    full = np.concatenate(outs, axis=0)  # [B, S]
    return full[:, None, :].astype(np.float32)


# revision 22
# speedup vs baseline: 4.6409x; 1.0591x over previous
"""Trainium2 Bass kernel for nn_Attn_25417616458107 (sparse_attention).

Reference computation:
    energy[s,b,:] = enc[s,b,:] @ W^T + b_attn          # [S,B,H]
    score[b,s]    = hidden[0,b,:] . energy[s,b,:]       # [B,S]
    out           = softmax(score, axis=s)[:, None, :]  # [B,1,S]

Key algebraic reformulation: reassociating the two contractions,
    score[b,s] = (hidden[0,b,:] @ W) . enc[s,b,:] + hidden[0,b,:].b_attn
The bias term is constant per row b, so it cancels in the softmax.  With
q = hidden[0] @ W (a tiny [B,H]x[H,H] matmul done on the host), the device
kernel reduces to a batched dot-product stream over encoder_outputs plus a
row softmax -- memory-bound instead of the naive 275-GFLOP einsum.

Sharding: data-parallel over batch.  Each of the 8 cores gets 8 of the 64
batches.  No cross-core communication.

Per core: 16 tiles, tile t covers s in [128t, 128t+128), s = 128t + 8*sa + sb.
SBUF tile [partition p=(b*16+sa), free f=(sb,h)].  The host pre-linearizes the
enc shard into exactly this [t, p, f] layout so every tile is one contiguous
4 MiB DMA (engages all 16 SDMA engines at ~395 GB/s/core; strided APs only
reached ~115).  Compute is a single fused DVE op per (tile, sb):
tensor_tensor_reduce does et*q2 (in place) + free-dim sum -> score column.
Scores bounce through DRAM (per tile, pipelined) to re-lay them as rows
[b, s] for the softmax (max / exp+sum via ACT accum / scale) and output DMA.
"""

import sys
import numpy as np

_S, _B, _H = 2048, 64, 1024
_NCORES = 8
_BLOC = _B // _NCORES  # 8 batches per core
_SA, _SB = 16, 8       # s = 128*t + 8*sa + sb; partition p = b*16+sa
_NT = _S // (_SA * _SB)  # 16 tiles

_cache = {}


def _concourse():
    if "/opt/trn_rl_repo" not in sys.path:
        sys.path.insert(0, "/opt/trn_rl_repo")


def _build():
    _concourse()
    import concourse.bacc as bacc
    import concourse.mybir as mybir
    import concourse.tile as tile

    f32 = mybir.dt.float32
    f16 = mybir.dt.float16
    nc = bacc.Bacc("TRN2", target_bir_lowering=False, debug=False)

    # enc/q2 staged in fp16: halves the HBM stream (the kernel's binding
    # resource) and enables the DVE 2x_1P perf mode for the multiply.
    # Scores accumulate in fp32; measured end-to-end rel err ~1.8e-3.
    enc = nc.dram_tensor("enc", [_NT, 128, _SB * _H], f16, kind="ExternalInput")
    q2 = nc.dram_tensor("q2", [128, _H], f16, kind="ExternalInput")
    out = nc.dram_tensor("out", [_BLOC, _S], f32, kind="ExternalOutput")
    scratch = nc.dram_tensor("scratch", [128, _NT * _SB], f32)

    # scratch[b*16+sa, t*8+sb] -> rows[b, s] with s = t*128 + sa*8 + sb,
    # bounced per-t (small DMAs) to stay within the 3-dim DMA AP limit.
    sc_cols = scratch.rearrange("p (t sb) -> t p sb", t=_NT)
    sc_rows = scratch.rearrange("(b sa) (t sb) -> t b sa sb", sa=_SA, t=_NT)

    with tile.TileContext(nc) as tc:
        with (
            tc.tile_pool(name="encp", bufs=5) as encp,
            tc.tile_pool(name="qp", bufs=1) as qp,
            tc.tile_pool(name="smallp", bufs=1) as smallp,
        ):
            q2t = qp.tile([128, _H], f16)
            nc.sync.dma_start(q2t[:], q2[:])

            scores = smallp.tile([128, _NT * _SB], f32)
            rows = smallp.tile([_BLOC, _S], f32)
            rows_t = rows.rearrange("b (t sa sb) -> t b sa sb", t=_NT, sa=_SA)

            for t in range(_NT):
                et = encp.tile([128, _SB * _H], f16, tag="enc")
                nc.sync.dma_start(et[:], enc[t])
                for sb in range(_SB):
                    sl = slice(sb * _H, (sb + 1) * _H)
                    # fused multiply + free-dim sum in one DVE op (in-place
                    # product; the fp32 accum_out column is the score)
                    nc.vector.scalar_tensor_tensor(
                        out=et[:, sl],
                        in0=et[:, sl],
                        scalar=1.0,
                        in1=q2t[:],
                        op0=mybir.AluOpType.mult,
                        op1=mybir.AluOpType.mult,
                        accum_out=scores[:, t * _SB + sb : t * _SB + sb + 1],
                    )
                # pipelined bounce out: tile t's score columns -> DRAM, issued on
                # the ACT HWDGE ring (its wait is satisfied by ACT program order,
                # so it never stalls; SP ring stays a pure enc stream; SWDGE
                # would drag SDMA engines 7/15 via descriptor-ring port traffic).
                nc.scalar.dma_start(sc_cols[t], scores[:, t * _SB : (t + 1) * _SB])

            # bounce back in: re-lay scores as rows[b, s]; all cols landed long ago
            for t in range(_NT):
                nc.sync.dma_start(rows_t[t], sc_rows[t])

            negmx = smallp.tile([_BLOC, 1], f32)
            nc.vector.tensor_reduce(
                negmx[:],
                rows[:],
                axis=mybir.AxisListType.X,
                op=mybir.AluOpType.max,
                negate=True,
            )
            erows = smallp.tile([_BLOC, _S], f32)
            zsum = smallp.tile([_BLOC, 1], f32)
            nc.scalar.activation(
                erows[:],
                rows[:],
                mybir.ActivationFunctionType.Exp,
                bias=negmx[:],
                scale=1.0,
                accum_out=zsum[:],
            )
            rz = smallp.tile([_BLOC, 1], f32)
            nc.vector.reciprocal(rz[:], zsum[:])
            nc.vector.tensor_scalar_mul(erows[:], erows[:], rz[:])
            nc.sync.dma_start(out[:], erows[:])

    nc.compile()
    return nc


def _in_maps(hidden, encoder_outputs, W_attn):
    hidden = np.asarray(hidden, dtype=np.float32)
    enc = np.asarray(encoder_outputs, dtype=np.float32)
    W = np.asarray(W_attn, dtype=np.float32)
    q = hidden[0] @ W  # [B, H]; bias term is constant per row -> cancels in softmax
    maps = []
    for c in range(_NCORES):
        bsl = slice(c * _BLOC, (c + 1) * _BLOC)
        q2 = np.ascontiguousarray(np.repeat(q[bsl], _SA, axis=0), dtype=np.float16)
        # linearize the shard into the exact on-chip tile layout [t, p, f]
        enc_lin = np.ascontiguousarray(
            enc[:, bsl, :]
            .reshape(_NT, _SA, _SB, _BLOC, _H)  # t, sa, sb, b, h
            .transpose(0, 3, 1, 2, 4)           # t, b, sa, sb, h
            .reshape(_NT, 128, _SB * _H)
            .astype(np.float16)
        )
        maps.append({"enc": enc_lin, "q2": q2})
    return maps


def kernel(hidden, encoder_outputs, W_attn, b_attn, **_unused):
    _concourse()
    from concourse.bass_utils import run_bass_kernel_spmd

    if "nc" not in _cache:
        _cache["nc"] = _build()
    nc = _cache["nc"]

    maps = _in_maps(hidden, encoder_outputs, W_attn)
    res = run_bass_kernel_spmd(nc, maps, core_ids=list(range(_NCORES)))
    outs = [np.asarray(res.results[c]["out"]) for c in range(_NCORES)]
    full = np.concatenate(outs, axis=0)  # [B, S]
    return full[:, None, :].astype(np.float32)


# revision 24
# speedup vs baseline: 5.5780x; 1.2019x over previous
"""Trainium2 Bass kernel for nn_Attn_25417616458107 (sparse_attention).

Reference computation:
    energy[s,b,:] = enc[s,b,:] @ W^T + b_attn          # [S,B,H]
    score[b,s]    = hidden[0,b,:] . energy[s,b,:]       # [B,S]
    out           = softmax(score, axis=s)[:, None, :]  # [B,1,S]

Key algebraic reformulation: reassociating the two contractions,
    score[b,s] = (hidden[0,b,:] @ W) . enc[s,b,:] + hidden[0,b,:].b_attn
The bias term is constant per row b, so it cancels in the softmax.  With
q = hidden[0] @ W (a tiny [B,H]x[H,H] matmul done on the host), the device
kernel reduces to a batched dot-product stream over encoder_outputs plus a
row softmax -- memory-bound instead of the naive 275-GFLOP einsum.

Sharding: data-parallel over batch.  Each of the 8 cores gets 8 of the 64
batches.  No cross-core communication.

Per core: 16 tiles, tile t covers s in [128t, 128t+128), s = 128t + 8*sa + sb.
SBUF tile [partition p=(b*16+sa), free f=(sb,h)].  The host pre-linearizes the
enc shard into exactly this [t, p, f] layout so every tile is one contiguous
4 MiB DMA (engages all 16 SDMA engines at ~395 GB/s/core; strided APs only
reached ~115).  Compute is a single fused DVE op per (tile, sb):
tensor_tensor_reduce does et*q2 (in place) + free-dim sum -> score column.
Scores bounce through DRAM (per tile, pipelined) to re-lay them as rows
[b, s] for the softmax (max / exp+sum via ACT accum / scale) and output DMA.
"""

import sys
import numpy as np

_S, _B, _H = 2048, 64, 1024
_NCORES = 8
_BLOC = _B // _NCORES  # 8 batches per core
_SA, _SB = 16, 8       # s = 128*t + 8*sa + sb; partition p = b*16+sa
_NT = _S // (_SA * _SB)  # 16 tiles

_cache = {}


def _concourse():
    if "/opt/trn_rl_repo" not in sys.path:
        sys.path.insert(0, "/opt/trn_rl_repo")


def _build():
    _concourse()
    import concourse.bacc as bacc
    import concourse.mybir as mybir
    import concourse.tile as tile

    f32 = mybir.dt.float32
    f16 = mybir.dt.float16
    nc = bacc.Bacc("TRN2", target_bir_lowering=False, debug=False)

    # enc/q2 staged in fp16: halves the HBM stream (the kernel's binding
    # resource) and enables the DVE 2x_1P perf mode for the multiply.
    # Scores accumulate in fp32; measured end-to-end rel err ~1.8e-3.
    enc = nc.dram_tensor("enc", [_NT, 128, _SB * _H], f16, kind="ExternalInput")
    q2 = nc.dram_tensor("q2", [128, _SB * _H], f16, kind="ExternalInput")
    out = nc.dram_tensor("out", [_BLOC, _S], f32, kind="ExternalOutput")
    scratch = nc.dram_tensor("scratch", [128, _NT * _SB], f32)

    # scratch[b*16+sa, t*8+sb] -> rows[b, s] with s = t*128 + sa*8 + sb,
    # bounced per-t (small DMAs) to stay within the 3-dim DMA AP limit.
    sc_cols = scratch.rearrange("p (t sb) -> t p sb", t=_NT)
    sc_rows = scratch.rearrange("(b sa) (t sb) -> t b sa sb", sa=_SA, t=_NT)

    with tile.TileContext(nc) as tc:
        with (
            tc.tile_pool(name="encp", bufs=5) as encp,
            tc.tile_pool(name="qp", bufs=1) as qp,
            tc.tile_pool(name="dumpp", bufs=2) as dumpp,
            tc.tile_pool(name="smallp", bufs=1) as smallp,
        ):
            q2t = qp.tile([128, _SB * _H], f16)
            nc.sync.dma_start(q2t[:], q2[:])

            scores = smallp.tile([128, _NT * _SB], f32)
            rows = smallp.tile([_BLOC, _S], f32)
            rows_t = rows.rearrange("b (t sa sb) -> t b sa sb", t=_NT, sa=_SA)

            for t in range(_NT):
                et = encp.tile([128, _SB * _H], f16, tag="enc")
                nc.sync.dma_start(et[:], enc[t])
                # Measured HW costs per [128,1024] fp16 slice: plain TT mult
                # runs at 2x (600ns), ACT copy-with-accum 1100ns (1x), fused
                # scalar_tensor_tensor 1302ns (1x).  Splitting the 8 slices --
                # k fused on DVE, the rest as one big 2x TT + per-slice ACT
                # accums -- balances DVE ~101us vs ACT ~97us (k alternates 2/3).
                k = 2 + (t % 2)
                # one 2x-mode multiply for slices k..7 (in place), q2 repeated
                nc.vector.tensor_mul(
                    et[:, k * _H :],
                    et[:, k * _H :],
                    q2t[:, k * _H :],
                )
                for sb in range(k, _SB):
                    dump = dumpp.tile([128, _H], f16, tag="dump")
                    nc.scalar.activation(
                        dump[:],
                        et[:, sb * _H : (sb + 1) * _H],
                        mybir.ActivationFunctionType.Copy,
                        accum_out=scores[:, t * _SB + sb : t * _SB + sb + 1],
                    )
                for sb in range(k):
                    sl = slice(sb * _H, (sb + 1) * _H)
                    nc.vector.scalar_tensor_tensor(
                        out=et[:, sl],
                        in0=et[:, sl],
                        scalar=1.0,
                        in1=q2t[:, sl],
                        op0=mybir.AluOpType.mult,
                        op1=mybir.AluOpType.mult,
                        accum_out=scores[:, t * _SB + sb : t * _SB + sb + 1],
                    )
                # pipelined bounce out: tile t's score columns -> DRAM, issued on
                # the ACT HWDGE ring (its wait is satisfied by ACT program order,
                # so it never stalls; SP ring stays a pure enc stream; SWDGE
                # would drag SDMA engines 7/15 via descriptor-ring port traffic).
                nc.scalar.dma_start(sc_cols[t], scores[:, t * _SB : (t + 1) * _SB])

            # bounce back in: re-lay scores as rows[b, s]; all cols landed long ago
            for t in range(_NT):
                nc.sync.dma_start(rows_t[t], sc_rows[t])

            negmx = smallp.tile([_BLOC, 1], f32)
            nc.vector.tensor_reduce(
                negmx[:],
                rows[:],
                axis=mybir.AxisListType.X,
                op=mybir.AluOpType.max,
                negate=True,
            )
            erows = smallp.tile([_BLOC, _S], f32)
            zsum = smallp.tile([_BLOC, 1], f32)
            nc.scalar.activation(
                erows[:],
                rows[:],
                mybir.ActivationFunctionType.Exp,
                bias=negmx[:],
                scale=1.0,
                accum_out=zsum[:],
            )
            rz = smallp.tile([_BLOC, 1], f32)
            nc.vector.reciprocal(rz[:], zsum[:])
            nc.vector.tensor_scalar_mul(erows[:], erows[:], rz[:])
            nc.sync.dma_start(out[:], erows[:])

    nc.compile()
    return nc


def _in_maps(hidden, encoder_outputs, W_attn):
    hidden = np.asarray(hidden, dtype=np.float32)
    enc = np.asarray(encoder_outputs, dtype=np.float32)
    W = np.asarray(W_attn, dtype=np.float32)
    q = hidden[0] @ W  # [B, H]; bias term is constant per row -> cancels in softmax
    maps = []
    for c in range(_NCORES):
        bsl = slice(c * _BLOC, (c + 1) * _BLOC)
        q2 = np.ascontiguousarray(
            np.tile(np.repeat(q[bsl], _SA, axis=0), (1, _SB)), dtype=np.float16
        )  # [128, SB*H]: q row repeated across all sb slices
        # linearize the shard into the exact on-chip tile layout [t, p, f]
        enc_lin = np.ascontiguousarray(
            enc[:, bsl, :]
            .reshape(_NT, _SA, _SB, _BLOC, _H)  # t, sa, sb, b, h
            .transpose(0, 3, 1, 2, 4)           # t, b, sa, sb, h
            .reshape(_NT, 128, _SB * _H)
            .astype(np.float16)
        )
        maps.append({"enc": enc_lin, "q2": q2})
    return maps


def kernel(hidden, encoder_outputs, W_attn, b_attn, **_unused):
    _concourse()
    from concourse.bass_utils import run_bass_kernel_spmd

    if "nc" not in _cache:
        _cache["nc"] = _build()
    nc = _cache["nc"]

    maps = _in_maps(hidden, encoder_outputs, W_attn)
    res = run_bass_kernel_spmd(nc, maps, core_ids=list(range(_NCORES)))
    outs = [np.asarray(res.results[c]["out"]) for c in range(_NCORES)]
    full = np.concatenate(outs, axis=0)  # [B, S]
    return full[:, None, :].astype(np.float32)


# revision 25
# speedup vs baseline: 5.7959x; 1.0391x over previous
"""Trainium2 Bass kernel for nn_Attn_25417616458107 (sparse_attention).

Reference computation:
    energy[s,b,:] = enc[s,b,:] @ W^T + b_attn          # [S,B,H]
    score[b,s]    = hidden[0,b,:] . energy[s,b,:]       # [B,S]
    out           = softmax(score, axis=s)[:, None, :]  # [B,1,S]

Key algebraic reformulation: reassociating the two contractions,
    score[b,s] = (hidden[0,b,:] @ W) . enc[s,b,:] + hidden[0,b,:].b_attn
The bias term is constant per row b, so it cancels in the softmax.  With
q = hidden[0] @ W (a tiny [B,H]x[H,H] matmul done on the host), the device
kernel reduces to a batched dot-product stream over encoder_outputs plus a
row softmax -- memory-bound instead of the naive 275-GFLOP einsum.

Sharding: data-parallel over batch.  Each of the 8 cores gets 8 of the 64
batches.  No cross-core communication.

Per core: 16 tiles, tile t covers s in [128t, 128t+128), s = 128t + 8*sa + sb.
SBUF tile [partition p=(b*16+sa), free f=(sb,h)].  The host pre-linearizes the
enc shard into exactly this [t, p, f] layout so every tile is one contiguous
4 MiB DMA (engages all 16 SDMA engines at ~395 GB/s/core; strided APs only
reached ~115).  Compute is a single fused DVE op per (tile, sb):
tensor_tensor_reduce does et*q2 (in place) + free-dim sum -> score column.
Scores bounce through DRAM (per tile, pipelined) to re-lay them as rows
[b, s] for the softmax (max / exp+sum via ACT accum / scale) and output DMA.
"""

import sys
import numpy as np

_S, _B, _H = 2048, 64, 1024
_NCORES = 8
_BLOC = _B // _NCORES  # 8 batches per core
_SA, _SB = 16, 8       # s = 128*t + 8*sa + sb; partition p = b*16+sa
_NT = _S // (_SA * _SB)  # 16 tiles

_cache = {}


def _concourse():
    if "/opt/trn_rl_repo" not in sys.path:
        sys.path.insert(0, "/opt/trn_rl_repo")


def _build():
    _concourse()
    import concourse.bacc as bacc
    import concourse.mybir as mybir
    import concourse.tile as tile

    f32 = mybir.dt.float32
    f16 = mybir.dt.float16
    nc = bacc.Bacc("TRN2", target_bir_lowering=False, debug=False)

    # enc/q2 staged in fp16: halves the HBM stream (the kernel's binding
    # resource) and enables the DVE 2x_1P perf mode for the multiply.
    # Scores accumulate in fp32; measured end-to-end rel err ~1.8e-3.
    enc = nc.dram_tensor("enc", [_NT, 128, _SB * _H], f16, kind="ExternalInput")
    q2 = nc.dram_tensor("q2", [128, _SB * _H], f16, kind="ExternalInput")
    out = nc.dram_tensor("out", [_BLOC, _S], f32, kind="ExternalOutput")
    scratch = nc.dram_tensor("scratch", [128, _NT * _SB], f32)

    # scratch[b*16+sa, t*8+sb] -> rows[b, s] with s = t*128 + sa*8 + sb,
    # bounced per-t (small DMAs) to stay within the 3-dim DMA AP limit.
    sc_cols = scratch.rearrange("p (t sb) -> t p sb", t=_NT)
    sc_rows = scratch.rearrange("(b sa) (t sb) -> t b sa sb", sa=_SA, t=_NT)

    with tile.TileContext(nc) as tc:
        with (
            tc.tile_pool(name="encp", bufs=5) as encp,
            tc.tile_pool(name="qp", bufs=1) as qp,
            tc.tile_pool(name="dumpp", bufs=2) as dumpp,
            tc.tile_pool(name="smallp", bufs=1) as smallp,
        ):
            q2t = qp.tile([128, _SB * _H], f16)
            nc.sync.dma_start(q2t[:], q2[:])

            scores = smallp.tile([128, _NT * _SB], f32)
            rows = smallp.tile([_BLOC, _S], f32)
            rows_t = rows.rearrange("b (t sa sb) -> t b sa sb", t=_NT, sa=_SA)

            for t in range(_NT):
                et = encp.tile([128, _SB * _H], f16, tag="enc")
                nc.sync.dma_start(et[:], enc[t])
                # Measured HW costs per [128,1024] fp16 slice: plain TT mult
                # 2x (600ns incl. marginal ~533), ACT copy-with-accum 1165ns +
                # 290ns accumulator drain, fused scalar_tensor_tensor ~1384ns.
                # k fused slices on DVE + one big 2x TT + ACT accums for the
                # rest balances DVE ~124us vs ACT ~121us (k alternates 3/4).
                k = 3 + (t % 2)
                # one 2x-mode multiply for slices k..7 (in place), q2 repeated
                nc.vector.tensor_mul(
                    et[:, k * _H :],
                    et[:, k * _H :],
                    q2t[:, k * _H :],
                )
                for sb in range(k, _SB):
                    dump = dumpp.tile([128, _H], f16, tag="dump")
                    nc.scalar.activation(
                        dump[:],
                        et[:, sb * _H : (sb + 1) * _H],
                        mybir.ActivationFunctionType.Copy,
                        accum_out=scores[:, t * _SB + sb : t * _SB + sb + 1],
                    )
                for sb in range(k):
                    sl = slice(sb * _H, (sb + 1) * _H)
                    nc.vector.scalar_tensor_tensor(
                        out=et[:, sl],
                        in0=et[:, sl],
                        scalar=1.0,
                        in1=q2t[:, sl],
                        op0=mybir.AluOpType.mult,
                        op1=mybir.AluOpType.mult,
                        accum_out=scores[:, t * _SB + sb : t * _SB + sb + 1],
                    )
                # pipelined bounce out on GPSIMD's SWDGE ring: keeps both the SP
                # enc stream and the saturated ACT free of DMA issue + waits.
                # (SWDGE descriptor traffic drags SDMA engines 7/15 a little,
                # but the fp16 stream leaves them ~50% idle.)
                nc.gpsimd.dma_start(sc_cols[t], scores[:, t * _SB : (t + 1) * _SB])

            # bounce back in: re-lay scores as rows[b, s]; all cols landed long ago
            for t in range(_NT):
                nc.sync.dma_start(rows_t[t], sc_rows[t])

            negmx = smallp.tile([_BLOC, 1], f32)
            nc.vector.tensor_reduce(
                negmx[:],
                rows[:],
                axis=mybir.AxisListType.X,
                op=mybir.AluOpType.max,
                negate=True,
            )
            erows = smallp.tile([_BLOC, _S], f32)
            zsum = smallp.tile([_BLOC, 1], f32)
            nc.scalar.activation(
                erows[:],
                rows[:],
                mybir.ActivationFunctionType.Exp,
                bias=negmx[:],
                scale=1.0,
                accum_out=zsum[:],
            )
            rz = smallp.tile([_BLOC, 1], f32)
            nc.vector.reciprocal(rz[:], zsum[:])
            nc.vector.tensor_scalar_mul(erows[:], erows[:], rz[:])
            nc.sync.dma_start(out[:], erows[:])

    nc.compile()
    return nc


def _in_maps(hidden, encoder_outputs, W_attn):
    hidden = np.asarray(hidden, dtype=np.float32)
    enc = np.asarray(encoder_outputs, dtype=np.float32)
    W = np.asarray(W_attn, dtype=np.float32)
    q = hidden[0] @ W  # [B, H]; bias term is constant per row -> cancels in softmax
    maps = []
    for c in range(_NCORES):
        bsl = slice(c * _BLOC, (c + 1) * _BLOC)
        q2 = np.ascontiguousarray(
            np.tile(np.repeat(q[bsl], _SA, axis=0), (1, _SB)), dtype=np.float16
        )  # [128, SB*H]: q row repeated across all sb slices
        # linearize the shard into the exact on-chip tile layout [t, p, f]
        enc_lin = np.ascontiguousarray(
            enc[:, bsl, :]
            .reshape(_NT, _SA, _SB, _BLOC, _H)  # t, sa, sb, b, h
            .transpose(0, 3, 1, 2, 4)           # t, b, sa, sb, h
            .reshape(_NT, 128, _SB * _H)
            .astype(np.float16)
        )
        maps.append({"enc": enc_lin, "q2": q2})
    return maps


def kernel(hidden, encoder_outputs, W_attn, b_attn, **_unused):
    _concourse()
    from concourse.bass_utils import run_bass_kernel_spmd

    if "nc" not in _cache:
        _cache["nc"] = _build()
    nc = _cache["nc"]

    maps = _in_maps(hidden, encoder_outputs, W_attn)
    res = run_bass_kernel_spmd(nc, maps, core_ids=list(range(_NCORES)))
    outs = [np.asarray(res.results[c]["out"]) for c in range(_NCORES)]
    full = np.concatenate(outs, axis=0)  # [B, S]
    return full[:, None, :].astype(np.float32)
